# revision 15
# baseline (speedup 1.0000x reference)
"""PlanarConsistencyLoss on 8 TRN2 NeuronCores.

total = sum_j |points_j . n[a_j] + d[a_j]| over valid j (a_j >= 0),
return total / count (or total if count == 0).

Strategy: pure data-parallel over B (one batch per core). Per core:
  - host packs a 65-entry parameter table [zeros; (n_p, d_p)] (entry 0
    catches invalid/padded points so they contribute exactly 0),
  - device gathers per-point plane params with GPSIMD InstIndirectCopy
    (table replicated on all 128 partitions; per-16-partition-group
    shared index lists, fed chunk by chunk),
  - a consolidation DMA redistributes the (16x redundant) gather output
    into a dense [128, TC, 4] tile matching the point layout,
  - DVE computes dist = |x.n + d| and running sums; GPSIMD reduces
    across partitions; host sums the 8 per-core (sum, count) pairs.

Raw Bass (no Tile) with standalone semaphore waits: the IndirectCopy
ISA struct accepts at most one attached sync wait, so all ordering is
done with explicit wait_ge instructions.
"""
import sys

for _p in ("/opt/trn_rl_repo", "/root/.axon_site/_ro/trn_rl_repo"):
    if _p not in sys.path:
        sys.path.insert(0, _p)

from contextlib import ExitStack

import numpy as np

import concourse.bass as bass
from concourse import mybir
from concourse import bass_isa
from concourse.bass_utils import run_bass_kernel_spmd

B, N, P = 8, 500000, 64
NENT = P + 1          # table entries (entry 0 = invalid/pad)
W = 4                 # f32 words per table entry: nx, ny, nz, d

# full-size pipeline config
NCH_FULL, TC_FULL = 21, 192     # 21 chunks x 192 cols; 128*4032 = 516096 padded pts
# (IC dst elem count 16*192*4 = 12288 per partition is known-good; 12544 fails
#  the compiler's has_valid_s4d4_ic_dst_elem_count check)

AOP = mybir.AluOpType
DT = mybir.dt


def _indirect_copy_flat(gp, out_flat, data_flat, idxs, num_valid):
    """InstIndirectCopy with flat 2D APs (the ISA-valid form the compiler
    accepts; bass's indirect_copy helper emits 3D APs that fail the
    has_valid_s4d4_ic_dst_elem_count codegen check)."""
    return gp.add_instruction(
        mybir.InstIndirectCopy(
            name=f"I-{gp.bass.next_id()}",
            ins=[gp.lower_ap(data_flat), gp.lower_ap(idxs)],
            outs=[gp.lower_ap(out_flat)],
            num_valid_indices=num_valid,
        )
    )


def build_bass(nch=NCH_FULL, tc=TC_FULL):
    """Build the single-core program (run SPMD on 8 cores)."""
    cols = nch * tc               # code/point columns per partition
    chpts = 16 * tc               # points gathered per chunk per group
    nc = bass.Bass("TRN2", target_bir_lowering=False, debug=False)

    tab_d = nc.dram_tensor("table", [128, NENT * W], DT.float32, kind="ExternalInput")
    pts_d = nc.dram_tensor("points", [128, cols * 3], DT.float32, kind="ExternalInput")
    a32_d = nc.dram_tensor("a32", [128, cols], DT.int32, kind="ExternalInput")
    out_d = nc.dram_tensor("out", [128, 2], DT.float32, kind="ExternalOutput")

    with ExitStack() as ctx:
        e = ctx.enter_context
        tab = e(nc.sbuf_tensor("tab", [128, NENT * W], DT.float32))
        pt = e(nc.sbuf_tensor("pt", [128, cols, 3], DT.float32))
        a32 = e(nc.sbuf_tensor("a32sb", [128, cols], DT.int32))
        codes = e(nc.sbuf_tensor("codes", [128, cols], DT.uint16))
        gout = [
            e(nc.sbuf_tensor(f"gout{i}", [128, chpts, W], DT.float32))
            for i in range(2)
        ]
        prm = [
            e(nc.sbuf_tensor(f"prm{i}", [128, tc, W], DT.float32)) for i in range(2)
        ]
        prods = e(nc.sbuf_tensor("prods", [128, tc, 3], DT.float32))
        dots = e(nc.sbuf_tensor("dots", [128, tc], DT.float32))
        dist = e(nc.sbuf_tensor("dist", [128, tc], DT.float32))
        partials = e(nc.sbuf_tensor("partials", [128, nch], DT.float32))
        dsc = e(nc.sbuf_tensor("dsc", [128, 2], DT.float32))

        s_tab = e(nc.semaphore("s_tab"))
        s_a32 = e(nc.semaphore("s_a32"))
        s_pts = e(nc.semaphore("s_pts"))
        s_codes = e(nc.semaphore("s_codes"))
        s_ic = e(nc.semaphore("s_ic"))
        s_cons = [e(nc.semaphore(f"s_cons{i}")) for i in range(2)]
        s_dve = e(nc.semaphore("s_dve"))
        s_fin = e(nc.semaphore("s_fin"))

        block = e(nc.Block())

        @block.sync
        def _(sync):
            sync.dma_start(tab[:], tab_d.ap()).then_inc(s_tab, 16)
            sync.dma_start(a32[:], a32_d.ap()).then_inc(s_a32, 16)
            sync.dma_start(
                pt[:].rearrange("p t c -> p (t c)"), pts_d.ap()
            ).then_inc(s_pts, 16)
            sync.wait_ge(s_dve, nch + 1)
            sync.dma_start(out_d.ap(), dsc[:]).then_inc(s_fin, 16)
            sync.wait_ge(s_fin, 16)

        @block.scalar
        def _(scalar):
            # consolidation DMAs on the ACT HWDGE queue
            for k in range(nch):
                b = k % 2
                scalar.wait_ge(s_ic, k + 1)
                if k >= 2:
                    scalar.wait_ge(s_dve, k - 1)
                for g in range(8):
                    scalar.dma_start(
                        prm[b][16 * g:16 * g + 16, :, :],
                        gout[b][16 * g:16 * g + 1, :, :],
                    ).then_inc(s_cons[b], 16)

        @block.gpsimd
        def _(gpsimd):
            gpsimd.wait_ge(s_tab, 16)
            gpsimd.wait_ge(s_codes, 1)
            nic = tc // 16           # ICs per chunk (dst cap: 1024 elems)
            for k in range(nch):
                b = k % 2
                if k >= 2:
                    gpsimd.wait_ge(s_cons[b], 128 * (k // 2))
                gflat = gout[b][:].rearrange("p a b -> p (a b)")
                for j in range(nic):
                    ic = _indirect_copy_flat(
                        gpsimd,
                        gflat[:, j * 256 * W:(j + 1) * 256 * W],
                        tab[:],
                        codes[:, k * tc + j * 16:k * tc + (j + 1) * 16],
                        256,
                    )
                ic.then_inc(s_ic, 1)

        @block.vector
        def _(vector):
            vector.wait_ge(s_a32, 16)
            # codes = (a + 1) * W  (uint16 element offsets into the table)
            vector.tensor_scalar(codes[:], a32[:], 1, W, AOP.add, AOP.mult)
            # count valid (a >= 0) -> dsc[:, 1]; scratch output into gout[0]
            cnt_scratch = gout[0][:].rearrange("p a b -> p (a b)")[:, 0:cols]
            vector.tensor_scalar(cnt_scratch, a32[:], 0, None, AOP.is_ge)
            vector.drain()
            vector.tensor_reduce(
                dsc[:, 1:2], cnt_scratch, mybir.AxisListType.X, AOP.add
            )
            vector.drain().then_inc(s_codes, 1)
            vector.wait_ge(s_pts, 16)  # points loaded
            for k in range(nch):
                b = k % 2
                vector.wait_ge(s_cons[b], 128 * (k // 2 + 1))
                ptk = pt[:, k * tc:(k + 1) * tc, :]
                vector.tensor_tensor(prods[:], ptk, prm[b][:, :, 0:3], AOP.mult)
                vector.drain()
                vector.tensor_reduce(dots[:], prods[:], mybir.AxisListType.X, AOP.add)
                vector.drain()
                prmd = prm[b][:, :, 3:4].rearrange("p t o -> p (t o)")
                vector.tensor_tensor(dist[:], dots[:], prmd, AOP.add)
                vector.drain()
                vector.scalar_tensor_tensor(
                    prods[:].rearrange("p t c -> p (t c)")[:, 0:tc],
                    dist[:], -1.0, dist[:], AOP.mult, AOP.max,
                    accum_out=partials[:, k:k + 1],
                )
                vector.drain().then_inc(s_dve, 1)
            vector.tensor_reduce(
                dsc[:, 0:1], partials[:], mybir.AxisListType.X, AOP.add
            )
            vector.drain().then_inc(s_dve, 1)

    return nc


def host_prep(points_b, assign_b, normals_b, offsets_b, nch=NCH_FULL, tc=TC_FULL):
    """Lay out one batch's inputs for the device (pure marshaling)."""
    cols = nch * tc
    npad = 128 * cols
    ppg = npad // 8          # points per group
    chb = 16 * tc            # points per chunk per group

    n = points_b.shape[0]
    pts = np.zeros((npad, 3), dtype=np.float32)
    pts[:n] = points_b
    a = np.full((npad,), -1, dtype=np.int32)
    a[:n] = assign_b

    # PT[16g+s, k*tc+j] = point[g*ppg + k*chb + s*tc + j]
    ptt = pts.reshape(8, nch, 16, tc, 3).transpose(0, 2, 1, 3, 4)
    ptt = np.ascontiguousarray(ptt).reshape(128, cols * 3)
    # CT[16g+s, k*tc+t] = a[g*ppg + k*chb + 16*t + s]
    at = a.reshape(8, nch, tc, 16).transpose(0, 3, 1, 2)
    at = np.ascontiguousarray(at).reshape(128, cols)

    tabe = np.zeros((NENT, W), dtype=np.float32)
    tabe[1:, 0:3] = normals_b
    tabe[1:, 3] = offsets_b
    tab = np.ascontiguousarray(
        np.broadcast_to(tabe.reshape(1, NENT * W), (128, NENT * W))
    )
    return {"table": tab, "points": ptt, "a32": at}


_NC_CACHE = {}


def kernel(points, plane_normals, plane_offsets, plane_assignments, _trace=False):
    key = (NCH_FULL, TC_FULL)
    if key not in _NC_CACHE:
        _NC_CACHE[key] = build_bass()
    nc = _NC_CACHE[key]

    points = np.asarray(points, dtype=np.float32)
    plane_normals = np.asarray(plane_normals, dtype=np.float32)
    plane_offsets = np.asarray(plane_offsets, dtype=np.float32)
    assign = np.asarray(plane_assignments).astype(np.int32)

    in_maps = [
        host_prep(points[b], assign[b], plane_normals[b], plane_offsets[b])
        for b in range(B)
    ]
    r = run_bass_kernel_spmd(nc, in_maps, list(range(B)), trace=_trace)
    outs = np.stack([r.results[i]["out"] for i in range(B)])  # (8, 128, 2)
    total = np.float64(outs[:, :, 0].sum())
    count = np.float64(outs[:, :, 1].sum())
    res = np.float32(total / count) if count > 0 else np.float32(total)
    if _trace:
        return res, r
    return res
